# revision 3
# baseline (speedup 1.0000x reference)
# Fused dynamic-conv (CondInst-style) + dice loss kernel for 8x TRN2 NeuronCores.
# v3: pixel-sliced cores, dense phase-C, full-width single matmuls per phase.
#
# Reference computation (per batch image b, object o):
#   weight[b,o,:] = conv_weight[b, :, ind[b,o]]           (gather, 593 params)
#   feat = concat(seg_feat[b], x_rel(o), y_rel(o))        ([18, 128*128])
#   h1 = relu(w1 @ feat + b1); h2 = relu(w2 @ h1 + b2)    (16-ch dynamic 1x1 convs)
#   out = sigmoid(w3 . h2 + b3)                           ([128*128])
#   dice over masked objects -> scalar loss
#
# Strategy:
#  * Host gathers the 593 dynamic params per active object and packs groups of
#    8 objects (block-diagonal weights).  x_rel/y_rel are separable, so the
#    object offsets fold into an effective bias b1_eff and one shared
#    [18, px] feature map per image serves all its objects.
#  * Sharding: each of the 8 cores takes a 2048-px slice of the image plane
#    and processes ALL groups (~8) over its slice.  gemm3 outputs from all
#    groups per px-tile-pair accumulate into one fully dense [128, 512] PSUM
#    bank (partition = 64*tile_parity + 8*group + obj), so sigmoid/dice ops
#    touch 4x fewer columns than a per-group layout.
#  * Each gemm is ONE full-width matmul per 512-px tile (M=128): gemm1
#    lhsT [18,128], gemm2 block-diagonal lhsT [128,128], gemm3 lhsT [128,64]
#    accumulating across groups.  PE cost is column-streaming bound either
#    way (512 cycles/tile/layer), but single matmuls keep the PE queue free
#    of LDWEIGHTS churn and need no feat band replication in SBUF.
#  * Evacuations (PSUM->SBUF relu+bias) split ~17:15 between ACT and DVE
#    (the only PSUM readers; both run ~1x for fp32 PSUM reads - this is the
#    hard floor of the kernel at ~20us/core).
#  * Emission is software-pipelined: step k emits A(k), B(k-1), evacuations,
#    C(k-2), so the PE queue never head-of-line blocks on an evacuation and
#    the HAM clock-gate stays open.
#  * Each pair's phase-C chain opens with a zero-weight K=1 matmul that
#    clears the whole bank's has_written bits; the 2*G real gemm3 matmuls
#    then accumulate (start=False) into their 64-partition halves.
#  * Dice partials: sigmoid+Square on ACT, sum(pred*tgt) on DVE, per
#    px-pair; host does the final tiny reduction plus sum(tgt^2), which is
#    network-independent.
import numpy as np
from contextlib import ExitStack

import concourse.bass as bass
import concourse.tile as tile
from concourse import mybir, bacc
from concourse.bass_utils import run_bass_kernel_spmd

C = 16
WT = 593
B, O, H, W = 4, 32, 128, 128
HW = H * W
N_CORES = 8
GRP = 8                  # objects per block-diagonal group
PX = HW // N_CORES       # pixels per core (2048)
NT = PX // 512           # 512-px moving tiles per core (4)
NPAIR = NT // 2          # px-tile pairs per core (2)

F32 = mybir.dt.float32
F16 = mybir.dt.float16
ACTF = mybir.ActivationFunctionType
ALU = mybir.AluOpType

# per-group packed weight columns in wcat: [0:64] lhsT3, [64:192] lhsT2,
# [192:320] lhsT1 (rows 0:18)
WCOLS = 320


def host_pack(seg_feat, conv_weight, mask, ind, target):
    cw = conv_weight.reshape(B, WT, HW)
    weight = np.take_along_axis(cw, ind[:, None, :].astype(np.int64), axis=2)
    weight = np.ascontiguousarray(weight.transpose(0, 2, 1))  # [B, O, WT]
    s0 = (C + 2) * C
    w1 = weight[..., :s0].reshape(B, O, C, C + 2)
    b1 = weight[..., s0:s0 + C]
    w2 = weight[..., s0 + C:s0 + C + C * C].reshape(B, O, C, C)
    b2 = weight[..., s0 + C + C * C:s0 + 2 * C + C * C]
    w3 = weight[..., s0 + 2 * C + C * C:s0 + 3 * C + C * C]
    b3 = weight[..., -1]
    xo = (ind % W).astype(np.float32)
    yo = (ind // W).astype(np.float32)

    # global group list: (image, [8 objects padded with -1])
    groups = []
    for b in range(B):
        objs = [o for o in range(O) if mask[b, o] == 1]
        for g0 in range(0, len(objs), GRP):
            grp = objs[g0:g0 + GRP]
            groups.append((b, grp + [-1] * (GRP - len(grp))))
    G = len(groups)
    S = (G + 7) // 8                     # group-sets of <= 8
    img_map = tuple(b for b, _ in groups)

    wpack = np.zeros((128, WCOLS * G), np.float16)
    biasg = np.zeros((128, 2 * G), np.float32)    # col 2g b1_eff, 2g+1 b2
    b3p = np.full((128, S), -50.0, np.float32)    # pred-layout bias
    for g, (b, grp) in enumerate(groups):
        st, gs = g // 8, g % 8
        c0 = WCOLS * g
        for oo, o in enumerate(grp):
            if o < 0:
                continue
            # lhsT1 [126, 128]: 7 replicated 18-row bands of w1^T/7 (the
            # feat is replicated 7x in partitions so gemm1 uses 126 of the
            # 128 PE rows -- K=18 matmuls never open the HAM clock gate)
            w1r = (w1[b, o].T / 7.0).astype(np.float16)
            for rb in range(7):
                wpack[18 * rb:18 * rb + 18,
                      c0 + 192 + 16 * oo:c0 + 192 + 16 * oo + 16] = w1r
            b1e = (b1[b, o] - w1[b, o, :, 16] * (xo[b, o] / 128.0)
                   - w1[b, o, :, 17] * (yo[b, o] / 128.0))
            biasg[16 * oo:16 * oo + 16, 2 * g] = b1e
            # lhsT2 [128, 128] block-diagonal: block [16oo:16oo+16]^2 = w2^T
            wpack[16 * oo:16 * oo + 16,
                  c0 + 64 + 16 * oo:c0 + 64 + 16 * oo + 16] = \
                w2[b, o].T.astype(np.float16)
            biasg[16 * oo:16 * oo + 16, 2 * g + 1] = b2[b, o]
            # lhsT3 [128, 64]: col 8*gs+oo holds w3[o] at rows 16oo:16oo+16
            wpack[16 * oo:16 * oo + 16, c0 + 8 * gs + oo] = \
                w3[b, o].astype(np.float16)
            for th in range(2):
                b3p[64 * th + 8 * gs + oo, st] = b3[b, o]

    # per-core inputs
    px = np.arange(HW, dtype=np.float32)
    xg = (px % W) / 128.0
    yg = np.floor(px / W) / 128.0
    tgt_flat = target.reshape(B, O, HW)

    in_maps = []
    for ci in range(N_CORES):
        sl = slice(PX * ci, PX * (ci + 1))
        feat = np.zeros((B, 128, PX), np.float16)
        for b in range(B):
            one = np.empty((18, PX), np.float32)
            one[:16] = seg_feat[b].reshape(C, HW)[:, sl]
            one[16] = xg[sl]
            one[17] = yg[sl]
            feat[b, :126] = np.tile(one, (7, 1)).astype(np.float16)
        # tgt in pred layout: partition 64*th + 8*gs + oo, col 512*pair + j
        tgt = np.zeros((128, S * NPAIR * 512), np.float16)
        for g, (b, grp) in enumerate(groups):
            st, gs = g // 8, g % 8
            for oo, o in enumerate(grp):
                if o < 0:
                    continue
                for t in range(NT):
                    pair, th = t // 2, t % 2
                    g0 = PX * ci + 512 * t
                    c0 = (st * NPAIR + pair) * 512
                    tgt[64 * th + 8 * gs + oo, c0:c0 + 512] = \
                        tgt_flat[b, o, g0:g0 + 512].astype(np.float16)
        in_maps.append({"feat": feat, "wpack": wpack, "biasg": biasg,
                        "b3p": b3p, "tgt": tgt})
    return in_maps, groups, G, S, img_map


_PROGRAM_CACHE = {}


def build_program(G, S, img_map):
    key = (G, S, img_map)
    if key in _PROGRAM_CACHE:
        return _PROGRAM_CACHE[key]
    nc = bacc.Bacc("TRN2", target_bir_lowering=False, debug=False,
                   enable_asserts=False, num_devices=N_CORES)
    feat_t = nc.dram_tensor("feat", (B, 128, PX), F16, kind="ExternalInput")
    wpack_t = nc.dram_tensor("wpack", (128, WCOLS * G), F16, kind="ExternalInput")
    biasg_t = nc.dram_tensor("biasg", (128, 2 * G), F32, kind="ExternalInput")
    b3p_t = nc.dram_tensor("b3p", (128, S), F32, kind="ExternalInput")
    tgt_t = nc.dram_tensor("tgt", (128, S * NPAIR * 512), F16, kind="ExternalInput")
    acc_t = nc.dram_tensor("acc", (128, 2 * S * NPAIR), F32, kind="ExternalOutput")

    with tile.TileContext(nc) as tc, ExitStack() as ctx:
        wpool = ctx.enter_context(tc.tile_pool(name="wpool", bufs=1))
        fpool = ctx.enter_context(tc.tile_pool(name="fpool", bufs=1))
        h1pool = ctx.enter_context(tc.tile_pool(name="h1pool", bufs=2))
        h2pool = ctx.enter_context(tc.tile_pool(name="h2pool", bufs=2))
        ppool = ctx.enter_context(tc.tile_pool(name="ppool", bufs=2))
        spool = ctx.enter_context(tc.tile_pool(name="spool", bufs=2))
        apool = ctx.enter_context(tc.tile_pool(name="apool", bufs=1))
        ps = ctx.enter_context(tc.tile_pool(name="ps", bufs=3, space="PSUM"))
        pcp = ctx.enter_context(tc.tile_pool(name="pcp", bufs=2, space="PSUM"))

        # col 2*(st*NPAIR+pair) = inter partial, col +1 = pred^2 partial
        # (interleaved so each pair's pair of columns can stream out in one
        # DMA as soon as that pair's dice ops complete)
        acc_sb = apool.tile([128, 2 * S * NPAIR], F32)

        # Warm-up during the initial DMA wait: trigger the sigmoid table set
        # (covers relu/square/sigmoid) and stream matmuls so the PE HAM
        # clock-gate opens before real work.  zlhs doubles as the zero
        # stationary operand that opens each phase-C accumulation chain.
        scr = apool.tile([128, 512], F16)
        nc.vector.memset(scr, 0.125)
        zlhs = apool.tile([1, 128], F16)
        nc.vector.memset(zlhs, 0.0)
        scr1 = apool.tile([128, 1], F32)
        nc.scalar.activation(scr1, scr[:, 0:1], ACTF.Sigmoid, bias=0.0, scale=1.0)
        pw = ps.tile([128, 1024], F32, tag="ps")
        # ~3.5us of warm-up matmuls: ends as the first input DMAs land, and
        # the HAM clock-gate (which needs ~3.4us of sustained PE activity)
        # opens right as real work begins
        for _ in range(11):
            nc.tensor.matmul(pw[:, 0:512], scr[:, 0:128], scr,
                             start=True, stop=True)

        # resident inputs; order matters: the sync/gpsimd DMA queues drain in
        # issue order at ~0.7-2us per transfer, so issue in first-use order.
        wcat = wpool.tile([128, WCOLS * G], F16, tag="w")
        bcat = wpool.tile([128, 2 * G], F32, tag="b")
        b3t = wpool.tile([128, S], F32, tag="b3")
        tgts = wpool.tile([128, 512 * NPAIR * S], F16, tag="t")
        ft = []
        for b in range(B):
            t_ = fpool.tile([128, PX], F16, tag=f"f{b}", name=f"ft{b}")
            ft.append(t_)
        gmid = (WCOLS * G) // 2
        # feat tiles split by partition half across the two DMA queues so
        # each image's features land in ~half the serial queue time
        nc.sync.dma_start(out=ft[0][0:64, :], in_=feat_t.ap()[0][0:64])
        nc.gpsimd.dma_start(out=ft[0][64:128, :], in_=feat_t.ap()[0][64:128])
        nc.sync.dma_start(out=wcat[:, 0:gmid], in_=wpack_t.ap()[:, 0:gmid])
        nc.gpsimd.dma_start(out=bcat, in_=biasg_t.ap())
        nc.sync.dma_start(out=ft[1][0:64, :], in_=feat_t.ap()[1][0:64])
        nc.gpsimd.dma_start(out=ft[1][64:128, :], in_=feat_t.ap()[1][64:128])
        nc.sync.dma_start(out=ft[2][0:64, :], in_=feat_t.ap()[2][0:64])
        nc.gpsimd.dma_start(out=ft[2][64:128, :], in_=feat_t.ap()[2][64:128])
        nc.sync.dma_start(out=wcat[:, gmid:], in_=wpack_t.ap()[:, gmid:])
        nc.gpsimd.dma_start(out=ft[3][64:128, :], in_=feat_t.ap()[3][64:128])
        nc.sync.dma_start(out=ft[3][0:64, :], in_=feat_t.ap()[3][0:64])
        nc.gpsimd.dma_start(out=b3t, in_=b3p_t.ap())
        nc.gpsimd.dma_start(out=tgts, in_=tgt_t.ap())

        evac_ctr = 0
        dice_cols = []

        def evac(dst, src, bias_ap):
            nonlocal evac_ctr
            i = evac_ctr
            # ~17/32 of evacuations on ACT, rest on DVE (balances the two
            # PSUM-reader engines: ACT ~1109ns, DVE ~1279ns per 1024 cols)
            on_act = ((i + 1) * 17) // 32 > (i * 17) // 32
            if on_act:
                nc.scalar.activation(dst, src, ACTF.Relu, bias=bias_ap, scale=1.0)
            else:
                nc.vector.tensor_scalar(out=dst, in0=src, scalar1=bias_ap,
                                        scalar2=0.0, op0=ALU.add, op1=ALU.max)
            evac_ctr += 1

        for st in range(S):
            gsl = list(range(8 * st, min(8 * st + 8, G)))
            ng = len(gsl)
            steps = [(pair, i) for pair in range(NPAIR) for i in range(ng)]
            n = len(steps)
            h1t = [None] * n
            h2t = [None] * n
            psA = [None] * n
            psB = [None] * n
            pct = [None] * NPAIR
            predt = [None] * NPAIR

            def do_A(k):
                pair, i = steps[k]
                g = gsl[i]
                c0 = WCOLS * g
                pa = ps.tile([128, 1024], F32, tag="ps", name=f"pa{k}")
                psA[k] = pa
                for s in range(2):
                    t = 2 * pair + s
                    nc.tensor.matmul(
                        pa[:, 512 * s:512 * s + 512],
                        wcat[0:126, c0 + 192:c0 + 320],
                        ft[img_map[g]][0:126, 512 * t:512 * t + 512],
                        start=True, stop=True)

            def do_evA(k):
                pair, i = steps[k]
                g = gsl[i]
                h1 = h1pool.tile([128, 1024], F16, tag="h1", name=f"h1_{k}")
                h1t[k] = h1
                evac(h1, psA[k], bcat[:, 2 * g:2 * g + 1])

            def do_B(k):
                pair, i = steps[k]
                g = gsl[i]
                c0 = WCOLS * g
                pb = ps.tile([128, 1024], F32, tag="ps", name=f"pb{k}")
                psB[k] = pb
                for s in range(2):
                    nc.tensor.matmul(
                        pb[:, 512 * s:512 * s + 512],
                        wcat[:, c0 + 64:c0 + 192],
                        h1t[k][:, 512 * s:512 * s + 512],
                        start=True, stop=True)

            def do_evB(k):
                pair, i = steps[k]
                g = gsl[i]
                h2 = h2pool.tile([128, 1024], F16, tag="h2", name=f"h2_{k}")
                h2t[k] = h2
                evac(h2, psB[k], bcat[:, 2 * g + 1:2 * g + 2])

            def do_C(k):
                pair, i = steps[k]
                g = gsl[i]
                c0 = WCOLS * g
                if i == 0:
                    pc = pcp.tile([128, 512], F32, tag="pc", name=f"pc{pair}")
                    pct[pair] = pc
                    # zero-weight matmul: clears has_written for the whole
                    # bank and writes exact zeros to all 128 partitions
                    nc.tensor.matmul(pc, zlhs, scr[0:1, 0:512],
                                     start=True, stop=False,
                                     skip_group_check=True)
                pc = pct[pair]
                for s in range(2):
                    nc.tensor.matmul(
                        pc[64 * s:64 * s + 64, :],
                        wcat[:, c0:c0 + 64],
                        h2t[k][:, 512 * s:512 * s + 512],
                        start=False, stop=(i == ng - 1 and s == 1),
                        tile_position=(0, 64 * s),
                        skip_group_check=True)

            def do_dice(pair):
                pred = ppool.tile([128, 512], F16, tag="pred", name=f"pred{pair}")
                predt[pair] = pred
                nc.scalar.activation(pred, pct[pair], ACTF.Sigmoid,
                                     bias=b3t[:, st:st + 1], scale=1.0)
                col = st * NPAIR + pair
                tsl = tgts[:, col * 512:col * 512 + 512]
                prod = spool.tile([128, 512], F16, tag="s", name=f"prod{pair}")
                nc.vector.scalar_tensor_tensor(
                    out=prod, in0=pred, scalar=0.0, in1=tsl,
                    op0=ALU.add, op1=ALU.mult,
                    accum_out=acc_sb[:, 2 * col:2 * col + 1])
                dice_cols.append(col)
                if pair == NPAIR - 1 and st == S - 1:
                    # final pair: pred^2 on ACT, in parallel with the DVE
                    # inter-product - shortens the serial kernel tail
                    sq = spool.tile([128, 512], F32, tag="sqf", name=f"sq{pair}")
                    nc.scalar.activation(sq, pred, ACTF.Square,
                                         accum_out=acc_sb[:, 2 * col + 1:2 * col + 2])
                else:
                    sq = spool.tile([128, 512], F16, tag="sq", name=f"sq{pair}")
                    nc.vector.scalar_tensor_tensor(
                        out=sq, in0=pred, scalar=0.0, in1=pred,
                        op0=ALU.add, op1=ALU.mult,
                        accum_out=acc_sb[:, 2 * col + 1:2 * col + 2])

            # pipelined emission: A(k) | B(k-1) | evacs | C(k-2)
            for k in range(n + 2):
                if k < n:
                    do_A(k)
                if 1 <= k <= n:
                    do_B(k - 1)
                if k < n:
                    do_evA(k)
                if 1 <= k <= n:
                    do_evB(k - 1)
                if 2 <= k:
                    kk = k - 2
                    do_C(kk)
                    if steps[kk][1] == ng - 1:
                        do_dice(steps[kk][0])
                        c = dice_cols[-1]
                        nc.sync.dma_start(
                            out=acc_t.ap()[:, 2 * c:2 * c + 2],
                            in_=acc_sb[:, 2 * c:2 * c + 2])


    nc.compile()
    _PROGRAM_CACHE[key] = nc
    return nc


def _run(inputs, trace=False):
    seg_feat = np.asarray(inputs["seg_feat"], np.float32)
    conv_weight = np.asarray(inputs["conv_weight"], np.float32)
    mask = np.asarray(inputs["mask"])
    ind = np.asarray(inputs["ind"])
    target = np.asarray(inputs["target"], np.float32)

    in_maps, groups, G, S, img_map = host_pack(
        seg_feat, conv_weight, mask, ind, target)
    nc = build_program(G, S, img_map)
    res = run_bass_kernel_spmd(nc, in_maps, core_ids=list(range(N_CORES)),
                               trace=trace)

    inter = np.zeros(B, np.float64)
    predsq = np.zeros(B, np.float64)
    for ci in range(N_CORES):
        acc_flat = res.results[ci]["acc"]  # [128, 2*S*NPAIR] interleaved
        acc = np.stack([acc_flat[:, 0::2], acc_flat[:, 1::2]])
        for g, (b, grp) in enumerate(groups):
            st, gs = g // 8, g % 8
            for oo, o in enumerate(grp):
                if o < 0:
                    continue
                for th in range(2):
                    q = 64 * th + 8 * gs + oo
                    inter[b] += acc[0, q, st * NPAIR:(st + 1) * NPAIR].sum(
                        dtype=np.float64)
                    predsq[b] += acc[1, q, st * NPAIR:(st + 1) * NPAIR].sum(
                        dtype=np.float64)
    tgtsq = ((target.reshape(B, O, HW).astype(np.float64) ** 2)
             * mask[:, :, None]).sum(axis=(1, 2))
    loss = 1.0 - (2.0 * inter + 1.0) / (predsq + tgtsq + 1.0)
    return np.float32(loss.mean()), res


def kernel(**inputs):
    loss, _ = _run(inputs, trace=False)
    return np.array(loss, dtype=np.float32)


# revision 4
# speedup vs baseline: 1.0113x; 1.0113x over previous
# Fused dynamic-conv (CondInst-style) + dice loss kernel for 8x TRN2 NeuronCores.
# v3: pixel-sliced cores, dense phase-C, full-width single matmuls per phase.
#
# Reference computation (per batch image b, object o):
#   weight[b,o,:] = conv_weight[b, :, ind[b,o]]           (gather, 593 params)
#   feat = concat(seg_feat[b], x_rel(o), y_rel(o))        ([18, 128*128])
#   h1 = relu(w1 @ feat + b1); h2 = relu(w2 @ h1 + b2)    (16-ch dynamic 1x1 convs)
#   out = sigmoid(w3 . h2 + b3)                           ([128*128])
#   dice over masked objects -> scalar loss
#
# Strategy:
#  * Host gathers the 593 dynamic params per active object and packs groups of
#    8 objects (block-diagonal weights).  x_rel/y_rel are separable, so the
#    object offsets fold into an effective bias b1_eff and one shared
#    [18, px] feature map per image serves all its objects.
#  * Sharding: each of the 8 cores takes a 2048-px slice of the image plane
#    and processes ALL groups (~8) over its slice.  gemm3 outputs from all
#    groups per px-tile-pair accumulate into one fully dense [128, 512] PSUM
#    bank (partition = 64*tile_parity + 8*group + obj), so sigmoid/dice ops
#    touch 4x fewer columns than a per-group layout.
#  * Each gemm is ONE full-width matmul per 512-px tile (M=128): gemm1
#    lhsT [18,128], gemm2 block-diagonal lhsT [128,128], gemm3 lhsT [128,64]
#    accumulating across groups.  PE cost is column-streaming bound either
#    way (512 cycles/tile/layer), but single matmuls keep the PE queue free
#    of LDWEIGHTS churn and need no feat band replication in SBUF.
#  * Evacuations (PSUM->SBUF relu+bias) split ~17:15 between ACT and DVE
#    (the only PSUM readers; both run ~1x for fp32 PSUM reads - this is the
#    hard floor of the kernel at ~20us/core).
#  * Emission is software-pipelined: step k emits A(k), B(k-1), evacuations,
#    C(k-2), so the PE queue never head-of-line blocks on an evacuation and
#    the HAM clock-gate stays open.
#  * Each pair's phase-C chain opens with a zero-weight K=1 matmul that
#    clears the whole bank's has_written bits; the 2*G real gemm3 matmuls
#    then accumulate (start=False) into their 64-partition halves.
#  * Dice partials: sigmoid+Square on ACT, sum(pred*tgt) on DVE, per
#    px-pair; host does the final tiny reduction plus sum(tgt^2), which is
#    network-independent.
import numpy as np
from contextlib import ExitStack

import concourse.bass as bass
import concourse.tile as tile
from concourse import mybir, bacc
from concourse.bass_utils import run_bass_kernel_spmd

C = 16
WT = 593
B, O, H, W = 4, 32, 128, 128
HW = H * W
N_CORES = 8
GRP = 8                  # objects per block-diagonal group
PX = HW // N_CORES       # pixels per core (2048)
NT = PX // 512           # 512-px moving tiles per core (4)
NPAIR = NT // 2          # px-tile pairs per core (2)

F32 = mybir.dt.float32
F16 = mybir.dt.float16
ACTF = mybir.ActivationFunctionType
ALU = mybir.AluOpType

# per-group packed weight columns in wcat: [0:64] lhsT3, [64:192] lhsT2,
# [192:320] lhsT1 (rows 0:18)
WCOLS = 320


def host_pack(seg_feat, conv_weight, mask, ind, target):
    cw = conv_weight.reshape(B, WT, HW)
    weight = np.take_along_axis(cw, ind[:, None, :].astype(np.int64), axis=2)
    weight = np.ascontiguousarray(weight.transpose(0, 2, 1))  # [B, O, WT]
    s0 = (C + 2) * C
    w1 = weight[..., :s0].reshape(B, O, C, C + 2)
    b1 = weight[..., s0:s0 + C]
    w2 = weight[..., s0 + C:s0 + C + C * C].reshape(B, O, C, C)
    b2 = weight[..., s0 + C + C * C:s0 + 2 * C + C * C]
    w3 = weight[..., s0 + 2 * C + C * C:s0 + 3 * C + C * C]
    b3 = weight[..., -1]
    xo = (ind % W).astype(np.float32)
    yo = (ind // W).astype(np.float32)

    # global group list: (image, [8 objects padded with -1])
    groups = []
    for b in range(B):
        objs = [o for o in range(O) if mask[b, o] == 1]
        for g0 in range(0, len(objs), GRP):
            grp = objs[g0:g0 + GRP]
            groups.append((b, grp + [-1] * (GRP - len(grp))))
    G = len(groups)
    S = (G + 7) // 8                     # group-sets of <= 8
    img_map = tuple(b for b, _ in groups)

    wpack = np.zeros((128, WCOLS * G), np.float16)
    biasg = np.zeros((128, 2 * G), np.float32)    # col 2g b1_eff, 2g+1 b2
    b3p = np.full((128, S), -50.0, np.float32)    # pred-layout bias
    for g, (b, grp) in enumerate(groups):
        st, gs = g // 8, g % 8
        c0 = WCOLS * g
        for oo, o in enumerate(grp):
            if o < 0:
                continue
            # lhsT1 [126, 128]: 7 replicated 18-row bands of w1^T/7 (the
            # feat is replicated 7x in partitions so gemm1 uses 126 of the
            # 128 PE rows -- K=18 matmuls never open the HAM clock gate)
            w1r = (w1[b, o].T / 7.0).astype(np.float16)
            for rb in range(7):
                wpack[18 * rb:18 * rb + 18,
                      c0 + 192 + 16 * oo:c0 + 192 + 16 * oo + 16] = w1r
            b1e = (b1[b, o] - w1[b, o, :, 16] * (xo[b, o] / 128.0)
                   - w1[b, o, :, 17] * (yo[b, o] / 128.0))
            biasg[16 * oo:16 * oo + 16, 2 * g] = b1e
            # lhsT2 [128, 128] block-diagonal: block [16oo:16oo+16]^2 = w2^T
            wpack[16 * oo:16 * oo + 16,
                  c0 + 64 + 16 * oo:c0 + 64 + 16 * oo + 16] = \
                w2[b, o].T.astype(np.float16)
            biasg[16 * oo:16 * oo + 16, 2 * g + 1] = b2[b, o]
            # lhsT3 [128, 64]: col 8*gs+oo holds w3[o] at rows 16oo:16oo+16
            wpack[16 * oo:16 * oo + 16, c0 + 8 * gs + oo] = \
                w3[b, o].astype(np.float16)
            for th in range(2):
                b3p[64 * th + 8 * gs + oo, st] = b3[b, o]

    # per-core inputs
    px = np.arange(HW, dtype=np.float32)
    xg = (px % W) / 128.0
    yg = np.floor(px / W) / 128.0
    tgt_flat = target.reshape(B, O, HW)

    in_maps = []
    for ci in range(N_CORES):
        sl = slice(PX * ci, PX * (ci + 1))
        feat = np.zeros((B, 128, PX), np.float16)
        for b in range(B):
            one = np.empty((18, PX), np.float32)
            one[:16] = seg_feat[b].reshape(C, HW)[:, sl]
            one[16] = xg[sl]
            one[17] = yg[sl]
            feat[b, :126] = np.tile(one, (7, 1)).astype(np.float16)
        # tgt in pred layout: partition 64*th + 8*gs + oo, col 512*pair + j
        tgt = np.zeros((128, S * NPAIR * 512), np.float16)
        for g, (b, grp) in enumerate(groups):
            st, gs = g // 8, g % 8
            for oo, o in enumerate(grp):
                if o < 0:
                    continue
                for t in range(NT):
                    pair, th = t // 2, t % 2
                    g0 = PX * ci + 512 * t
                    c0 = (st * NPAIR + pair) * 512
                    tgt[64 * th + 8 * gs + oo, c0:c0 + 512] = \
                        tgt_flat[b, o, g0:g0 + 512].astype(np.float16)
        in_maps.append({"feat": feat, "wpack": wpack, "biasg": biasg,
                        "b3p": b3p, "tgt": tgt})
    return in_maps, groups, G, S, img_map


_PROGRAM_CACHE = {}


def build_program(G, S, img_map):
    key = (G, S, img_map)
    if key in _PROGRAM_CACHE:
        return _PROGRAM_CACHE[key]
    nc = bacc.Bacc("TRN2", target_bir_lowering=False, debug=False,
                   enable_asserts=False, num_devices=N_CORES)
    feat_t = nc.dram_tensor("feat", (B, 128, PX), F16, kind="ExternalInput")
    wpack_t = nc.dram_tensor("wpack", (128, WCOLS * G), F16, kind="ExternalInput")
    biasg_t = nc.dram_tensor("biasg", (128, 2 * G), F32, kind="ExternalInput")
    b3p_t = nc.dram_tensor("b3p", (128, S), F32, kind="ExternalInput")
    tgt_t = nc.dram_tensor("tgt", (128, S * NPAIR * 512), F16, kind="ExternalInput")
    acc_t = nc.dram_tensor("acc", (128, 2 * S * NPAIR), F32, kind="ExternalOutput")

    with tile.TileContext(nc) as tc, ExitStack() as ctx:
        wpool = ctx.enter_context(tc.tile_pool(name="wpool", bufs=1))
        fpool = ctx.enter_context(tc.tile_pool(name="fpool", bufs=1))
        h1pool = ctx.enter_context(tc.tile_pool(name="h1pool", bufs=2))
        h2pool = ctx.enter_context(tc.tile_pool(name="h2pool", bufs=2))
        ppool = ctx.enter_context(tc.tile_pool(name="ppool", bufs=2))
        spool = ctx.enter_context(tc.tile_pool(name="spool", bufs=2))
        apool = ctx.enter_context(tc.tile_pool(name="apool", bufs=1))
        ps = ctx.enter_context(tc.tile_pool(name="ps", bufs=3, space="PSUM"))
        pcp = ctx.enter_context(tc.tile_pool(name="pcp", bufs=2, space="PSUM"))

        # col 2*(st*NPAIR+pair) = inter partial, col +1 = pred^2 partial
        # (interleaved so each pair's pair of columns can stream out in one
        # DMA as soon as that pair's dice ops complete)
        acc_sb = apool.tile([128, 2 * S * NPAIR], F32)

        # Warm-up during the initial DMA wait: trigger the sigmoid table set
        # (covers relu/square/sigmoid) and stream matmuls so the PE HAM
        # clock-gate opens before real work.  zlhs doubles as the zero
        # stationary operand that opens each phase-C accumulation chain.
        scr = apool.tile([128, 512], F16)
        nc.gpsimd.memset(scr, 0.125)
        zlhs = apool.tile([1, 128], F16)
        nc.gpsimd.memset(zlhs, 0.0)
        scr1 = apool.tile([128, 1], F32)
        nc.scalar.activation(scr1, scr[:, 0:1], ACTF.Sigmoid, bias=0.0, scale=1.0)
        pw = ps.tile([128, 1024], F32, tag="ps")
        # ~3.5us of warm-up matmuls: ends as the first input DMAs land, and
        # the HAM clock-gate (which needs ~3.4us of sustained PE activity)
        # opens right as real work begins
        for _ in range(11):
            nc.tensor.matmul(pw[:, 0:512], scr[:, 0:128], scr,
                             start=True, stop=True)

        # resident inputs; order matters: the sync/gpsimd DMA queues drain in
        # issue order at ~0.7-2us per transfer, so issue in first-use order.
        wcat = wpool.tile([128, WCOLS * G], F16, tag="w")
        bcat = wpool.tile([128, 2 * G], F32, tag="b")
        b3t = wpool.tile([128, S], F32, tag="b3")
        tgts = wpool.tile([128, 512 * NPAIR * S], F16, tag="t")
        ft = []
        for b in range(B):
            t_ = fpool.tile([128, PX], F16, tag=f"f{b}", name=f"ft{b}")
            ft.append(t_)
        gmid = (WCOLS * G) // 2
        # feat tiles split by partition half across the two DMA queues so
        # each image's features land in ~half the serial queue time
        nc.sync.dma_start(out=ft[0][0:64, :], in_=feat_t.ap()[0][0:64])
        nc.gpsimd.dma_start(out=ft[0][64:128, :], in_=feat_t.ap()[0][64:128])
        nc.sync.dma_start(out=wcat[:, 0:gmid], in_=wpack_t.ap()[:, 0:gmid])
        nc.gpsimd.dma_start(out=bcat, in_=biasg_t.ap())
        nc.sync.dma_start(out=ft[1][0:64, :], in_=feat_t.ap()[1][0:64])
        nc.gpsimd.dma_start(out=ft[1][64:128, :], in_=feat_t.ap()[1][64:128])
        nc.sync.dma_start(out=ft[2][0:64, :], in_=feat_t.ap()[2][0:64])
        nc.gpsimd.dma_start(out=ft[2][64:128, :], in_=feat_t.ap()[2][64:128])
        nc.sync.dma_start(out=wcat[:, gmid:], in_=wpack_t.ap()[:, gmid:])
        nc.gpsimd.dma_start(out=ft[3][64:128, :], in_=feat_t.ap()[3][64:128])
        nc.sync.dma_start(out=ft[3][0:64, :], in_=feat_t.ap()[3][0:64])
        nc.gpsimd.dma_start(out=b3t, in_=b3p_t.ap())
        nc.gpsimd.dma_start(out=tgts, in_=tgt_t.ap())

        evac_ctr = 0
        dice_cols = []

        def evac(dst, src, bias_ap):
            nonlocal evac_ctr
            i = evac_ctr
            # ~17/32 of evacuations on ACT, rest on DVE (balances the two
            # PSUM-reader engines: ACT ~1109ns, DVE ~1279ns per 1024 cols)
            on_act = ((i + 1) * 17) // 32 > (i * 17) // 32
            if on_act:
                nc.scalar.activation(dst, src, ACTF.Relu, bias=bias_ap, scale=1.0)
            else:
                nc.vector.tensor_scalar(out=dst, in0=src, scalar1=bias_ap,
                                        scalar2=0.0, op0=ALU.add, op1=ALU.max)
            evac_ctr += 1

        for st in range(S):
            gsl = list(range(8 * st, min(8 * st + 8, G)))
            ng = len(gsl)
            steps = [(pair, i) for pair in range(NPAIR) for i in range(ng)]
            n = len(steps)
            h1t = [None] * n
            h2t = [None] * n
            psA = [None] * n
            psB = [None] * n
            pct = [None] * NPAIR
            predt = [None] * NPAIR

            def do_A(k):
                pair, i = steps[k]
                g = gsl[i]
                c0 = WCOLS * g
                pa = ps.tile([128, 1024], F32, tag="ps", name=f"pa{k}")
                psA[k] = pa
                for s in range(2):
                    t = 2 * pair + s
                    nc.tensor.matmul(
                        pa[:, 512 * s:512 * s + 512],
                        wcat[0:126, c0 + 192:c0 + 320],
                        ft[img_map[g]][0:126, 512 * t:512 * t + 512],
                        start=True, stop=True)

            def do_evA(k):
                pair, i = steps[k]
                g = gsl[i]
                h1 = h1pool.tile([128, 1024], F16, tag="h1", name=f"h1_{k}")
                h1t[k] = h1
                evac(h1, psA[k], bcat[:, 2 * g:2 * g + 1])

            def do_B(k):
                pair, i = steps[k]
                g = gsl[i]
                c0 = WCOLS * g
                pb = ps.tile([128, 1024], F32, tag="ps", name=f"pb{k}")
                psB[k] = pb
                for s in range(2):
                    nc.tensor.matmul(
                        pb[:, 512 * s:512 * s + 512],
                        wcat[:, c0 + 64:c0 + 192],
                        h1t[k][:, 512 * s:512 * s + 512],
                        start=True, stop=True)

            def do_evB(k):
                pair, i = steps[k]
                g = gsl[i]
                h2 = h2pool.tile([128, 1024], F16, tag="h2", name=f"h2_{k}")
                h2t[k] = h2
                evac(h2, psB[k], bcat[:, 2 * g + 1:2 * g + 2])

            def do_C(k):
                pair, i = steps[k]
                g = gsl[i]
                c0 = WCOLS * g
                if i == 0:
                    pc = pcp.tile([128, 512], F32, tag="pc", name=f"pc{pair}")
                    pct[pair] = pc
                    # zero-weight matmul: clears has_written for the whole
                    # bank and writes exact zeros to all 128 partitions
                    nc.tensor.matmul(pc, zlhs, scr[0:1, 0:512],
                                     start=True, stop=False,
                                     skip_group_check=True)
                pc = pct[pair]
                for s in range(2):
                    nc.tensor.matmul(
                        pc[64 * s:64 * s + 64, :],
                        wcat[:, c0:c0 + 64],
                        h2t[k][:, 512 * s:512 * s + 512],
                        start=False, stop=(i == ng - 1 and s == 1),
                        tile_position=(0, 64 * s),
                        skip_group_check=True)

            def do_dice(pair):
                pred = ppool.tile([128, 512], F16, tag="pred", name=f"pred{pair}")
                predt[pair] = pred
                nc.scalar.activation(pred, pct[pair], ACTF.Sigmoid,
                                     bias=b3t[:, st:st + 1], scale=1.0)
                col = st * NPAIR + pair
                tsl = tgts[:, col * 512:col * 512 + 512]
                prod = spool.tile([128, 512], F16, tag="s", name=f"prod{pair}")
                nc.vector.scalar_tensor_tensor(
                    out=prod, in0=pred, scalar=0.0, in1=tsl,
                    op0=ALU.add, op1=ALU.mult,
                    accum_out=acc_sb[:, 2 * col:2 * col + 1])
                dice_cols.append(col)
                if pair == NPAIR - 1 and st == S - 1:
                    # final pair: pred^2 on ACT, in parallel with the DVE
                    # inter-product - shortens the serial kernel tail
                    sq = spool.tile([128, 512], F32, tag="sqf", name=f"sq{pair}")
                    nc.scalar.activation(sq, pred, ACTF.Square,
                                         accum_out=acc_sb[:, 2 * col + 1:2 * col + 2])
                else:
                    sq = spool.tile([128, 512], F16, tag="sq", name=f"sq{pair}")
                    nc.vector.scalar_tensor_tensor(
                        out=sq, in0=pred, scalar=0.0, in1=pred,
                        op0=ALU.add, op1=ALU.mult,
                        accum_out=acc_sb[:, 2 * col + 1:2 * col + 2])

            # pipelined emission: A(k) | B(k-1) | evacs | C(k-2)
            for k in range(n + 2):
                if k < n:
                    do_A(k)
                if 1 <= k <= n:
                    do_B(k - 1)
                if k < n:
                    do_evA(k)
                if 1 <= k <= n:
                    do_evB(k - 1)
                if 2 <= k:
                    kk = k - 2
                    do_C(kk)
                    if steps[kk][1] == ng - 1:
                        do_dice(steps[kk][0])
                        c = dice_cols[-1]
                        nc.sync.dma_start(
                            out=acc_t.ap()[:, 2 * c:2 * c + 2],
                            in_=acc_sb[:, 2 * c:2 * c + 2])


    nc.compile()
    _PROGRAM_CACHE[key] = nc
    return nc


def _run(inputs, trace=False):
    seg_feat = np.asarray(inputs["seg_feat"], np.float32)
    conv_weight = np.asarray(inputs["conv_weight"], np.float32)
    mask = np.asarray(inputs["mask"])
    ind = np.asarray(inputs["ind"])
    target = np.asarray(inputs["target"], np.float32)

    in_maps, groups, G, S, img_map = host_pack(
        seg_feat, conv_weight, mask, ind, target)
    nc = build_program(G, S, img_map)
    res = run_bass_kernel_spmd(nc, in_maps, core_ids=list(range(N_CORES)),
                               trace=trace)

    inter = np.zeros(B, np.float64)
    predsq = np.zeros(B, np.float64)
    for ci in range(N_CORES):
        acc_flat = res.results[ci]["acc"]  # [128, 2*S*NPAIR] interleaved
        acc = np.stack([acc_flat[:, 0::2], acc_flat[:, 1::2]])
        for g, (b, grp) in enumerate(groups):
            st, gs = g // 8, g % 8
            for oo, o in enumerate(grp):
                if o < 0:
                    continue
                for th in range(2):
                    q = 64 * th + 8 * gs + oo
                    inter[b] += acc[0, q, st * NPAIR:(st + 1) * NPAIR].sum(
                        dtype=np.float64)
                    predsq[b] += acc[1, q, st * NPAIR:(st + 1) * NPAIR].sum(
                        dtype=np.float64)
    tgtsq = ((target.reshape(B, O, HW).astype(np.float64) ** 2)
             * mask[:, :, None]).sum(axis=(1, 2))
    loss = 1.0 - (2.0 * inter + 1.0) / (predsq + tgtsq + 1.0)
    return np.float32(loss.mean()), res


def kernel(**inputs):
    loss, _ = _run(inputs, trace=False)
    return np.array(loss, dtype=np.float32)
